# revision 20
# baseline (speedup 1.0000x reference)
"""Causal self-attention (B=2, T=2048, C=1024, H=16) on 8 trn2 NeuronCores.

Sharding: tensor-parallel over heads — core c owns heads (2c, 2c+1).
Each core computes q/k/v for its 2 heads (128 of the 3072 c_attn output
features per projection), runs causal attention for its heads, and
produces a partial c_proj output (contraction over its 128 y-features).
Partials are summed on the host.

Device-side layouts (host pre-transposes/casts, all matmuls bf16 with fp32
PSUM accumulation):
  xT     [1024, 4096]  x^T (feature-major), shared by all cores
  wqkvT  [1024, 384]   per-core [q|k|v] weight columns; q/k feature order is
                       de-interleaved per head ([even d | odd d]) so RoPE's
                       even/odd pairing becomes a partition-block swap; the q
                       block is pre-scaled by 1/sqrt(hd)
  wpT    [128, 1024]   w_proj rows for this core's 128 y-features
  cos/sin [128, 2048]  RoPE tables replicated per feature row (sin rows carry
                       the rotation sign)
Scores are computed transposed (S^T[n,m]) so softmax needs no on-chip
transposes: exp without max-subtraction (scores are O(3) here), denominator
via a ones-column appended to V.  QKV token-tiles are interleaved between
attention tiles so the PE has work while ACT grinds through exp.
"""

import numpy as np
import ml_dtypes

B, T, C, H = 2, 2048, 1024, 16
HD = C // H          # 64
NT = B * T           # 4096
NCORES = 8
HPC = H // NCORES    # heads per core = 2
CPC = HPC * HD       # y-features per core = 128

TOK_TILE = 512       # moving-dim tile for qkv/proj matmuls and q-tiles
NJ = NT // TOK_TILE  # 8 token tiles
NCH = 128            # key-chunk size
NI = T // TOK_TILE   # 4 q-tiles per batch
BF16 = ml_dtypes.bfloat16

_CACHE = {}


def _split_waits(nc):
    """Cap sync waits at one per instruction.

    The walrus in this container rejects >1 sync-wait command on an
    instruction (seen for CTRL drains and DMA pseudo-instructions alike).
    Move all but the last wait of every instruction onto EventSemaphore
    instructions inserted just before it on the same engine.
    """
    import concourse.mybir as mybir

    n = 0
    for fn in nc.m.functions:
        for bb in fn.blocks:
            insts = bb.instructions
            out = []
            changed = False
            for inst in insts:
                si = inst.sync_info
                if si is not None and si.on_wait and len(si.on_wait) > 1:
                    waits = list(si.on_wait)
                    for w in waits[:-1]:
                        ev = mybir.InstEventSemaphore(
                            name=f"I-wsplit-{n}", ins=[], outs=[]
                        )
                        n += 1
                        ev.engine = inst.engine
                        ev.sync_info = mybir.SyncInfo(on_wait=[w], on_update=[])
                        out.append(ev)
                    si.on_wait = waits[-1:]
                    inst.sync_info = si
                    changed = True
                out.append(inst)
            if changed:
                bb.instructions = out


def _emit(nc, tc, ctx):
    import concourse.mybir as mybir
    from concourse.bass import AP as bass_AP
    from concourse.masks import make_identity

    DT = mybir.dt.bfloat16
    F32 = mybir.dt.float32
    Exp = mybir.ActivationFunctionType.Exp
    Copy = mybir.ActivationFunctionType.Copy

    xT_d = nc.declare_dram_parameter("xT", [C, NT], DT, isOutput=False)
    wqkvT_d = nc.declare_dram_parameter("wqkvT", [C, 3 * CPC], DT, isOutput=False)
    wpT_d = nc.declare_dram_parameter("wpT", [CPC, C], DT, isOutput=False)
    cos_d = nc.declare_dram_parameter("cos", [128, T], DT, isOutput=False)
    sin_d = nc.declare_dram_parameter("sin", [128, T], DT, isOutput=False)
    dmask_d = nc.declare_dram_parameter("dmask", [128, NCH], DT, isOutput=False)
    outT_d = nc.declare_dram_parameter("outT", [C, NT], DT, isOutput=True)

    const = ctx.enter_context(tc.tile_pool(name="const", bufs=1))
    xtp = ctx.enter_context(tc.tile_pool(name="xtp", bufs=2))
    work = ctx.enter_context(tc.tile_pool(name="work", bufs=5))
    esp = ctx.enter_context(tc.tile_pool(name="esp", bufs=28))
    stage = ctx.enter_context(tc.tile_pool(name="stage", bufs=2))
    psS = ctx.enter_context(tc.tile_pool(name="psS", bufs=2, space="PSUM"))
    psY = ctx.enter_context(tc.tile_pool(name="psY", bufs=2, space="PSUM"))
    psA = ctx.enter_context(tc.tile_pool(name="psA", bufs=2, space="PSUM"))

    # ---- persistent SBUF tensors; the first xt tiles and the qkv weights are
    # DMA'd ahead of everything else so the PE can start ASAP ----
    def load_xt(j, skip_dma=False):
        xt = xtp.tile([128, 8, TOK_TILE], DT, tag="xt", name=f"xt{j}")
        src = xT_d.rearrange("(a p) n -> p a n", p=128)[
            :, :, TOK_TILE * j : TOK_TILE * (j + 1)
        ]
        if not skip_dma:
            nc.sync.dma_start(out=xt, in_=src)
        return xt, src

    w_sb = const.tile([128, 8, 3 * CPC], DT, tag="w")
    w_src = wqkvT_d.rearrange("(a p) f -> p a f", p=128)
    cos_sb = const.tile([128, T], DT, tag="cos")
    sin_sb = const.tile([128, T], DT, tag="sin")
    # startup-latency ordering: j=0 is consumed per 128-feature chunk (q, k
    # and v matmuls per chunk) at DMA-arrival pace, so interleave per-chunk
    # weight+x loads; rope tables stream in just behind them
    xt0, xt0_src = load_xt(0, skip_dma=True)
    nc.sync.dma_start(out=w_sb[:, 0:1, :], in_=w_src[:, 0:1, :])
    nc.sync.dma_start(out=xt0[:, 0, :], in_=xt0_src[:, 0, :])
    nc.sync.dma_start(out=w_sb[:, 1:8, :], in_=w_src[:, 1:8, :])
    for ci in range(1, 8):
        nc.sync.dma_start(out=xt0[:, ci, :], in_=xt0_src[:, ci, :])
        if ci == 4:
            nc.sync.dma_start(out=cos_sb[:, 0:TOK_TILE], in_=cos_d[:, 0:TOK_TILE])
            nc.sync.dma_start(out=sin_sb[:, 0:TOK_TILE], in_=sin_d[:, 0:TOK_TILE])
    dmask_sb = const.tile([128, NCH], DT, tag="dmask")
    nc.sync.dma_start(out=dmask_sb, in_=dmask_d[:])
    xt4, _ = load_xt(4)
    nc.sync.dma_start(out=cos_sb[:, TOK_TILE:], in_=cos_d[:, TOK_TILE:])
    nc.sync.dma_start(out=sin_sb[:, TOK_TILE:], in_=sin_d[:, TOK_TILE:])
    ident = const.tile([128, 128], DT, tag="ident")
    make_identity(nc, ident)
    # warm the ACT exp table while phase 1 runs
    warm = const.tile([128, 1], F32, tag="warm")
    nc.vector.memset(warm, 0.0)
    nc.scalar.activation(warm, warm, Exp)
    wp_sb = const.tile([128, C], DT, tag="wp")
    nc.sync.dma_start(out=wp_sb, in_=wpT_d[:])

    qT_sb = const.tile([128, NT], DT, tag="qT")
    kT_sb = const.tile([128, NT], DT, tag="kT")
    yT_sb = const.tile([128, NT], DT, tag="yT")
    # v' [tok 128, chunk, head, hd+1] per global token chunk; col hd = 1.0
    vP = const.tile([128, NT // NCH, HPC, HD + 1], DT, tag="vP", name="vP")
    nc.vector.memset(vP[:, :, :, HD : HD + 1], 1.0)

    def rope(j, raw, jsl, csl):
        # rope on q|k at once: out = raw*cos + swap32(raw)*sin
        # (rotation sign folded into sin, 1/sqrt(hd) into wq)
        sw = work.tile([128, 2, TOK_TILE], DT, tag="sw", name=f"sw{j}", bufs=3)
        for a in range(4):
            sl = slice(32 * (a ^ 1), 32 * (a ^ 1) + 32)
            nc.sync.dma_start(out=sw[32 * a : 32 * a + 32], in_=raw[sl])
        cos2 = bass_AP(
            tensor=cos_sb.tensor,
            offset=cos_sb[:, csl].offset,
            ap=[cos_sb.ap[0], [0, 2], cos_sb[:, csl].ap[1]],
        )
        sin2 = bass_AP(
            tensor=sin_sb.tensor,
            offset=sin_sb[:, csl].offset,
            ap=[sin_sb.ap[0], [0, 2], sin_sb[:, csl].ap[1]],
        )
        nc.vector.tensor_mul(raw, raw, cos2)
        nc.vector.tensor_mul(sw, sw, sin2)
        nc.vector.tensor_add(qT_sb[:, jsl], raw[:, 0, :], sw[:, 0, :])
        nc.vector.tensor_add(kT_sb[:, jsl], raw[:, 1, :], sw[:, 1, :])

    def qkv_gen(j, xt=None, chunk_paced=False, rope_done=None):
        """Project tokens [512j, 512j+512): q/k (+rope) into qT/kT, v into vP
        (v is computed token-major directly: out[tok, feat], no transposes).
        Yields after each PE matmul so the caller can interleave this PE work
        into ACT-bound stretches of the attention pipeline.

        chunk_paced=True (startup, j=0): consume each 128-feature x chunk for
        q, k and v right as its DMA lands, accumulating q|k in a psS-pool
        tile (scores haven't started yet, so those banks are free)."""
        jsl = slice(TOK_TILE * j, TOK_TILE * (j + 1))
        csl = slice(TOK_TILE * (j % NI), TOK_TILE * (j % NI) + TOK_TILE)
        if xt is None:
            xt, _ = load_xt(j)
        raw = work.tile([128, 2, TOK_TILE], DT, tag="raw", name=f"raw{j}", bufs=3)
        accv = psA.tile([128, 4, HPC, HD], F32, tag="ps", name=f"v{j}")
        if chunk_paced:
            accqk = psS.tile([128, 2, TOK_TILE], F32, tag="ps", name=f"qk{j}")
            for ci in range(8):
                for f in range(2):
                    nc.tensor.matmul(
                        accqk[:, f, :],
                        lhsT=w_sb[:, ci, 128 * f : 128 * (f + 1)],
                        rhs=xt[:, ci, :],
                        start=(ci == 0),
                        stop=(ci == 7),
                    )
                    yield
                for s in range(4):
                    nc.tensor.matmul(
                        accv[:, s, :, :],
                        lhsT=xt[:, ci, 128 * s : 128 * s + 128],
                        rhs=w_sb[:, ci, 2 * CPC : 3 * CPC],
                        start=(ci == 0),
                        stop=(ci == 7),
                    )
                    yield
            nc.vector.tensor_copy(raw, accqk)
            rope(j, raw, jsl, csl)
            if rope_done is not None:
                rope_done[0] += 1
        else:
            for f in range(2):
                fsl = slice(128 * f, 128 * (f + 1))
                acc = psA.tile([128, TOK_TILE], F32, tag="ps", name=f"qkv{j}_{f}")
                for ci in range(8):
                    nc.tensor.matmul(
                        acc,
                        lhsT=w_sb[:, ci, fsl],
                        rhs=xt[:, ci, :],
                        start=(ci == 0),
                        stop=(ci == 7),
                    )
                    yield
                nc.vector.tensor_copy(raw[:, f, :], acc)
                if f == 1:
                    rope(j, raw, jsl, csl)
                    if rope_done is not None:
                        rope_done[0] += 1
            # v, token-major: out[tok, (h, d)] = sum_c x[c, tok] wv[c, (h, d)]
            for s in range(4):
                for ci in range(8):
                    nc.tensor.matmul(
                        accv[:, s, :, :],
                        lhsT=xt[:, ci, 128 * s : 128 * s + 128],
                        rhs=w_sb[:, ci, 2 * CPC : 3 * CPC],
                        start=(ci == 0),
                        stop=(ci == 7),
                    )
                    yield
        ch0 = (TOK_TILE * j) // NCH
        nc.vector.tensor_copy(vP[:, ch0 : ch0 + 4, :, 0:HD], accv)

    def s_gen(b, i, pairs):
        """Score chunks S^T[n, m] + exp for q-tile (b, i); fills `pairs` and
        yields after each chunk pair so the driver can spread this work."""
        tok0 = T * b
        m0 = tok0 + TOK_TILE * i
        msl = slice(m0, m0 + TOK_TILE)
        nch = 4 * (i + 1)
        for h in range(HPC):
            pairs.append([None] * (nch // 2))
        for u in range(nch // 2):
            for h in range(HPC):
                hsl = slice(HD * h, HD * (h + 1))
                ssp = psS.tile([128, 2 * TOK_TILE], F32, tag="ps", name=f"s{b}{i}{u}{h}")
                for idx in range(2):
                    jn = 2 * u + idx
                    n0 = tok0 + NCH * jn
                    # queries below the diagonal chunk are entirely masked:
                    # skip those columns in the score matmul too
                    c0 = NCH * max(jn - 4 * i, 0)
                    nc.tensor.matmul(
                        ssp[:, TOK_TILE * idx + c0 : TOK_TILE * (idx + 1)],
                        lhsT=kT_sb[hsl, n0 : n0 + NCH],
                        rhs=qT_sb[hsl, m0 + c0 : m0 + TOK_TILE],
                        start=True,
                        stop=True,
                    )
                es = esp.tile(
                    [128, 2 * TOK_TILE], DT, tag=f"es{h}", name=f"es{b}{i}{u}{h}"
                )
                k1 = 2 * u + 1 - 4 * i
                if k1 <= 0:
                    nc.scalar.activation(es, ssp, Exp)
                else:
                    for idx in range(2):
                        k = 2 * u + idx - 4 * i
                        c0 = TOK_TILE * idx + NCH * max(k, 0)
                        c1 = TOK_TILE * (idx + 1)
                        nc.scalar.activation(es[:, c0:c1], ssp[:, c0:c1], Exp)
                for idx in range(2):
                    k = 2 * u + idx - 4 * i
                    if k >= 0:
                        c0 = TOK_TILE * idx + NCH * k
                        # SBUF->SBUF, so this can ride the otherwise-idle
                        # gpsimd engine
                        nc.gpsimd.tensor_mul(
                            es[:, c0 : c0 + NCH], es[:, c0 : c0 + NCH], dmask_sb
                        )
                pairs[h][u] = es
            yield

    def emit_AV(b, i, pairs, pump_s, pump_q, pump_p):
        """attn @ v' for q-tile (b, i), normalize, transpose into yT.

        Both heads' AV go into one PSUM tile per half (sm pair) so the
        normalization is two batched DVE ops per half (reciprocal of the
        ones-column denominators + a broadcast multiply)."""
        tok0 = T * b
        m0 = tok0 + TOK_TILE * i
        ypks = []
        for half in range(2):
            yp = psY.tile([128, 2, HPC, HD + 1], F32, tag="ps", name=f"y{b}{i}{half}")
            for smh in range(2):
                sm = 2 * half + smh
                njn = 4 * i + sm + 1
                for h in range(HPC):
                    for jn in range(njn):
                        es = pairs[h][jn // 2]
                        base = TOK_TILE * (jn % 2)
                        nc.tensor.matmul(
                            yp[:, smh, h, :],
                            lhsT=es[:, base + NCH * sm : base + NCH * (sm + 1)],
                            rhs=vP[:, (tok0 // NCH) + jn, h, :],
                            start=(jn == 0),
                            stop=(jn == njn - 1),
                        )
                    pump_s(1)
                    pump_p(2)
                    pump_q(4)
            # normalization queues on DVE right behind this half's AV chains;
            # the other half's chains keep the PE fed meanwhile
            recip = work.tile([128, 2, HPC, 1], F32, tag="recip", name=f"rc{b}{i}{half}")
            nc.vector.reciprocal(recip, yp[:, :, :, HD : HD + 1])
            rbc = bass_AP(
                tensor=recip.tensor,
                offset=recip.offset,
                ap=[recip.ap[0], [HPC, 2], [1, HPC], [0, HD]],
            )
            ypk = work.tile([128, 2, HPC, HD], DT, tag="ypk", name=f"ypk{b}{i}{half}")
            nc.vector.tensor_mul(ypk, yp[:, :, :, 0:HD], rbc)
            ypks.append(ypk)
        for sm in range(4):
            pump_p(1)
            pump_q(2)
            pst = psA.tile([128, 128], DT, tag="ps", name=f"yt{b}{i}{sm}")
            nc.tensor.transpose(pst, ypks[sm // 2][:, sm % 2, :, :], ident)
            nc.vector.tensor_copy(
                yT_sb[:, m0 + NCH * sm : m0 + NCH * (sm + 1)], pst
            )

    def proj_gen(b, i, last):
        """c_proj slice for q-tile (b, i) plus its output DMA; deferred into a
        pumpable generator so it fills the PE during later tiles' exp waits."""
        m0 = T * b + TOK_TILE * i
        msl = slice(m0, m0 + TOK_TILE)
        for og in range(2):
            ost = stage.tile(
                [128, 4, TOK_TILE], DT, tag="ost", name=f"ost{b}{i}{og}"
            )
            for oi in range(4):
                ot = 4 * og + oi
                osp = psA.tile([128, TOK_TILE], F32, tag="ps", name=f"o{b}{i}{ot}")
                nc.tensor.matmul(
                    osp,
                    lhsT=wp_sb[:, 128 * ot : 128 * (ot + 1)],
                    rhs=yT_sb[:, msl],
                    start=True,
                    stop=True,
                )
                if ot == 0:
                    # a light sprinkle of PSUM-eviction work on ACT; the bulk
                    # stays on DVE (gpsimd cannot touch PSUM on trn2)
                    nc.scalar.activation(ost[:, oi, :], osp, Copy)
                else:
                    nc.vector.tensor_copy(ost[:, oi, :], osp)
                if last and og == 1:
                    # shorten the drain tail: per-slice DMAs so the final
                    # transfer is small
                    nc.sync.dma_start(
                        out=outT_d.rearrange("(a p) n -> p a n", p=128)[
                            :, 4 * og + oi : 4 * og + oi + 1, msl
                        ],
                        in_=ost[:, oi : oi + 1, :],
                    )
                yield
            if not (last and og == 1):
                nc.sync.dma_start(
                    out=outT_d.rearrange("(a p) n -> p a n", p=128)[
                        :, 4 * og : 4 * og + 4, msl
                    ],
                    in_=ost,
                )

    # ---- pipeline ----
    # Paced work queues are drip-fed between the attention matmuls of earlier
    # tiles: qkv matmul units (never stall; main PE filler), deferred c_proj
    # units of the previous tile (late-stage filler), and score chunk-pairs
    # for upcoming tiles (at most ~one per AV sub-chain, matching the rate
    # ACT drains them through exp — more would clog the in-order PE stream on
    # PSUM back-pressure).
    for _ in qkv_gen(0, xt0, chunk_paced=True):
        pass
    tiles = [(0, 0), (1, 0), (0, 1), (1, 1), (0, 2), (1, 2), (0, 3), (1, 3)]
    qkv_js = [4, 1, 5, 2, 6, 3, 7]
    rope_done = [1]  # count of qkv tiles whose q/k are roped (incl. eager j=0)
    qkv_gens = [qkv_gen(4, xt4, rope_done=rope_done)] + [
        qkv_gen(j, rope_done=rope_done) for j in qkv_js[1:]
    ]
    qkv_done = [1]  # count of fully-emitted qkv tiles (incl. eager j=0)
    pair_store = {t: [] for t in tiles}
    s_gens = [s_gen(b, i, pair_store[(b, i)]) for (b, i) in tiles]
    s_done = [0]  # count of fully-emitted s tiles
    proj_q = []  # deferred proj generators, drained as PE filler

    def pump_q(n):
        done = 0
        while qkv_done[0] <= len(qkv_js) and done < n:
            g = qkv_gens[qkv_done[0] - 1]
            try:
                next(g)
                done += 1
            except StopIteration:
                qkv_done[0] += 1

    def pump_s(n):
        # s tile k may only emit once the first k+1 qkv tiles are roped
        done = 0
        while s_done[0] < len(tiles) and done < n:
            k = s_done[0]
            if rope_done[0] < k + 1:
                pump_q(1)
                if rope_done[0] < k + 1:
                    return
                continue
            try:
                next(s_gens[k])
                done += 1
            except StopIteration:
                s_done[0] += 1

    def pump_p(n):
        done = 0
        while proj_q and done < n:
            try:
                next(proj_q[0])
                done += 1
            except StopIteration:
                proj_q.pop(0)

    for k, (b, i) in enumerate(tiles):
        while s_done[0] <= k:
            pump_s(1)
            pump_p(1)
            pump_q(1)
        # AV reads vP, whose writes for the needed key range must already be
        # in the instruction stream: tiles 1..k+1 fully emitted
        while qkv_done[0] < k + 1:
            pump_q(8)
        emit_AV(b, i, pair_store[(b, i)], pump_s, pump_q, pump_p)
        proj_q.append(proj_gen(b, i, last=(k == len(tiles) - 1)))
    while qkv_done[0] <= len(qkv_js):
        pump_q(100)
    pump_p(1000)


def _build_nc(split_waits=True):
    from contextlib import ExitStack

    import concourse.bass as bass
    import concourse.tile as tile

    nc = bass.Bass("TRN2", target_bir_lowering=False, debug=False, num_devices=NCORES)
    with ExitStack() as ctx:
        tc = ctx.enter_context(tile.TileContext(nc))
        _emit(nc, tc, ctx)
    if split_waits:
        # CoreSim's race detector can't digest the inserted EventSemaphores;
        # build with split_waits=False when simulating.
        _split_waits(nc)
    return nc


def _prep_inputs(x, w_attn, w_proj, freqs_cos, freqs_sin):
    x = np.asarray(x, np.float32)
    w_attn = np.asarray(w_attn, np.float32)
    w_proj = np.asarray(w_proj, np.float32)
    fc = np.asarray(freqs_cos, np.float32)
    fs = np.asarray(freqs_sin, np.float32)

    perm = np.concatenate([np.arange(0, HD, 2), np.arange(1, HD, 2)])
    xT = np.ascontiguousarray(x.reshape(NT, C).T).astype(BF16)

    pair = np.arange(128) % 32
    COS = fc[:, pair].T.copy()  # [128, T]
    SIN = fs[:, pair].T.copy()
    sign = np.where((np.arange(128) % 64) < 32, -1.0, 1.0).astype(np.float32)
    SIN = SIN * sign[:, None]
    scale = np.float32(1.0 / np.sqrt(HD))
    shared = {
        "xT": xT,
        "cos": COS.astype(BF16),
        "sin": SIN.astype(BF16),
        "dmask": (np.arange(NCH)[None, :] >= np.arange(128)[:, None]).astype(BF16),
    }

    in_maps = []
    for core in range(NCORES):
        heads = range(HPC * core, HPC * core + HPC)
        rows_q = np.concatenate([h * HD + perm for h in heads])
        rows_k = np.concatenate([C + h * HD + perm for h in heads])
        rows_v = np.concatenate([2 * C + h * HD + np.arange(HD) for h in heads])
        wqkvT = np.ascontiguousarray(
            np.concatenate(
                [w_attn[rows_q] * scale, w_attn[rows_k], w_attn[rows_v]], axis=0
            ).T
        ).astype(BF16)
        cols_v = np.concatenate([h * HD + np.arange(HD) for h in heads])
        wpT = np.ascontiguousarray(w_proj[:, cols_v].T).astype(BF16)
        in_maps.append({**shared, "wqkvT": wqkvT, "wpT": wpT})
    return in_maps


def _make_runner():
    """Compile the kernel once and return a reusable multi-core runner.

    Mirrors bass2jax.run_bass_via_pjrt's shard_map path, but keeps the jitted
    executable so repeat kernel() calls skip retracing/compile-cache lookups.
    """
    import jax
    import numpy as _np
    from jax.sharding import Mesh, PartitionSpec
    from jax.experimental.shard_map import shard_map

    import concourse.mybir as mybir
    from concourse import bass2jax

    nc = _build_nc()
    bass2jax.install_neuronx_cc_hook()

    partition_name = nc.partition_id_tensor.name if nc.partition_id_tensor else None
    in_names, out_names, out_avals, zero_shapes = [], [], [], []
    for alloc in nc.m.functions[0].allocations:
        if not isinstance(alloc, mybir.MemoryLocationSet):
            continue
        name = alloc.memorylocations[0].name
        if alloc.kind == "ExternalInput":
            if name != partition_name:
                in_names.append(name)
        elif alloc.kind == "ExternalOutput":
            shape = tuple(alloc.tensor_shape)
            dtype = mybir.dt.np(alloc.dtype)
            out_names.append(name)
            out_avals.append(jax.core.ShapedArray(shape, dtype))
            zero_shapes.append((shape, dtype))
    n_params = len(in_names)
    n_outs = len(out_avals)
    all_names = in_names + out_names + ([partition_name] if partition_name else [])
    donate = tuple(range(n_params, n_params + n_outs))

    def _body(*args):
        operands = list(args)
        if partition_name is not None:
            operands.append(bass2jax.partition_id_tensor())
        outs = bass2jax._bass_exec_p.bind(
            *operands,
            out_avals=tuple(out_avals),
            in_names=tuple(all_names),
            out_names=tuple(out_names),
            lowering_input_output_aliases=(),
            sim_require_finite=True,
            sim_require_nnan=True,
            nc=nc,
        )
        return tuple(outs)

    devices = jax.devices()[:NCORES]
    mesh = Mesh(_np.asarray(devices), ("core",))
    sharded = jax.jit(
        shard_map(
            _body,
            mesh=mesh,
            in_specs=(PartitionSpec("core"),) * (n_params + n_outs),
            out_specs=(PartitionSpec("core"),) * n_outs,
            check_rep=False,
        ),
        donate_argnums=donate,
        keep_unused=True,
    )

    def run(in_maps):
        concat_in = [
            np.concatenate([np.asarray(m[name]) for m in in_maps], axis=0)
            for name in in_names
        ]
        concat_zeros = [
            np.zeros((NCORES * s[0], *s[1:]), dt) for (s, dt) in zero_shapes
        ]
        out_arrs = sharded(*concat_in, *concat_zeros)
        return [
            {
                name: np.asarray(out_arrs[i]).reshape(
                    NCORES, *out_avals[i].shape
                )[c]
                for i, name in enumerate(out_names)
            }
            for c in range(NCORES)
        ]

    return run


def kernel(x, w_attn, w_proj, freqs_cos, freqs_sin):
    if "runner" not in _CACHE:
        _CACHE["runner"] = _make_runner()
    in_maps = _prep_inputs(x, w_attn, w_proj, freqs_cos, freqs_sin)
    results = _CACHE["runner"](in_maps)
    out = np.zeros((C, NT), np.float64)
    for r in results:
        out += r["outT"].astype(np.float64)
    return out.T.reshape(B, T, C).astype(np.float32)

